# revision 1
# baseline (speedup 1.0000x reference)
"""Bayesian linear layer sampling kernel for 8 Trainium2 NeuronCores.

Reference computation (per batch b, sample s):
    L = strictly_lower(cov_flat) + diag(exp(logvar/2))        # [N, N], N=2080
    W[b,s,:] = loc + eps[b,s,:] @ L.T                          # [B,S,N]
    y[b,s,k] = sum_h x[b,s,h] * W[b,s, 32+32h+k] + W[b,s,k]    # [B,S,32]

Strategy:
  - Data parallel over the batch dim: one batch of 2048 samples per core.
  - The loc contribution is affine in x only:
        y0[s,k] = loc[k] + sum_h x[s,h]*loc[32+32h+k]
    so it is precomputed on host and the device only computes
        Wg = eps @ L^T    (the 17.7 GFLOP/core GEMM)
        y  = y0 + Wg[:, :32] + sum_h x[:,h] * Wg[:, 32+32h:64+32h]
  - GEMM on the tensor engine with contraction dim n on partitions:
    stationary lhsT = eps^T tiles [128n x 128s] (eps transposed on host,
    zero padded from 2080 to 2176 = 17*128 rows), moving rhs = L^T tiles
    [128n x 416m].  dtype float32r -> 1 cycle/row on TRN2 for free dim
    >= 256 (full bf16-rate fp32 matmul).
  - m split into 5 chunks of 416 (all >=256, and 416 = 13*32 keeps the
    32-wide per-h weight blocks unsplit across chunks).
  - Epilogue per 128-sample tile: 65 fused multiply-accumulate ops on the
    vector engine reading W straight from PSUM:
        y = (psum_block * x[:,h]) + y     (scalar_tensor_tensor)
"""

import sys

if "/opt/trn_rl_repo" not in sys.path:
    sys.path.insert(0, "/opt/trn_rl_repo")

import numpy as np

IN_F, OUT_F = 64, 32
N = IN_F * OUT_F + OUT_F          # 2080
B, S = 8, 2048
P = 128
NT = 17                           # n tiles (2080 padded to 2176 = 17*128)
NPAD = NT * P
M_CHUNK = 416                     # 5 chunks of 416 = 2080, each 13 blocks of 32
N_CHUNKS = N // M_CHUNK
S_BLK = 256                       # eps^T DMA slab width in s (2 s-tiles)
N_CORES = 8


def build_module(use_f32r: bool = True, n_stiles: int = S // P):
    """Build the Bass module (one NeuronCore's program, run SPMD on 8)."""
    import concourse.bass as bass  # noqa: F401
    import concourse.mybir as mybir
    from concourse import bacc, tile

    nc = bacc.Bacc("TRN2", target_bir_lowering=False, debug=False,
                   num_devices=N_CORES)
    mmdt = mybir.dt.float32r if use_f32r else mybir.dt.float32
    f32 = mybir.dt.float32

    epsT = nc.dram_tensor("epsT", [NPAD, S], mmdt, kind="ExternalInput")
    LT = nc.dram_tensor("LT", [NPAD, N], mmdt, kind="ExternalInput")
    x = nc.dram_tensor("x", [S, IN_F], f32, kind="ExternalInput")
    y0 = nc.dram_tensor("y0", [S, OUT_F], f32, kind="ExternalInput")
    y = nc.dram_tensor("y", [S, OUT_F], f32, kind="ExternalOutput")

    mult = mybir.AluOpType.mult
    add = mybir.AluOpType.add

    with tile.TileContext(nc) as tc:
        with (
            tc.tile_pool(name="lt", bufs=NT) as lt_pool,
            tc.tile_pool(name="eps", bufs=2 * NT) as eps_pool,
            tc.tile_pool(name="xy", bufs=3) as xy_pool,
            tc.tile_pool(name="psum", bufs=4, space="PSUM") as psum_pool,
        ):
            # L^T resident in SBUF for the whole kernel (reused by every
            # s-tile): 17 tiles x [128, 2080] = 141 KB/partition.
            lt_sb = []
            for nt in range(NT):
                t = lt_pool.tile([P, N], mmdt, tag="lt")
                nc.sync.dma_start(out=t[:], in_=LT[nt * P:(nt + 1) * P, :])
                lt_sb.append(t)

            n_sblocks = (n_stiles * P + S_BLK - 1) // S_BLK
            for sb in range(n_sblocks):
                # eps^T slab for S_BLK samples: 17 tiles [128n, S_BLK s];
                # 1 KB contiguous per partition per DMA.
                eps_sb = []
                for nt in range(NT):
                    t = eps_pool.tile([P, S_BLK], mmdt, tag="eps")
                    nc.sync.dma_start(
                        out=t[:],
                        in_=epsT[nt * P:(nt + 1) * P,
                                 sb * S_BLK:(sb + 1) * S_BLK],
                    )
                    eps_sb.append(t)

                for half in range(S_BLK // P):
                    st = sb * (S_BLK // P) + half
                    if st >= n_stiles:
                        break
                    s0 = st * P
                    x_sb = xy_pool.tile([P, IN_F], f32, tag="x")
                    nc.sync.dma_start(out=x_sb[:], in_=x[s0:s0 + P, :])
                    y0_sb = xy_pool.tile([P, OUT_F], f32, tag="y0")
                    nc.sync.dma_start(out=y0_sb[:], in_=y0[s0:s0 + P, :])
                    y_sb = xy_pool.tile([P, OUT_F], f32, tag="y")

                    for c in range(N_CHUNKS):
                        m0 = c * M_CHUNK
                        ps = psum_pool.tile([P, M_CHUNK], f32, tag="ps")
                        for nt in range(NT):
                            nc.tensor.matmul(
                                ps[:],
                                lhsT=eps_sb[nt][:, half * P:(half + 1) * P],
                                rhs=lt_sb[nt][:, m0:m0 + M_CHUNK],
                                start=(nt == 0),
                                stop=(nt == NT - 1),
                            )
                        # epilogue for this chunk's 32-wide h blocks
                        if c == 0:
                            # bias block: y = Wg[:, 0:32] + y0
                            nc.vector.tensor_tensor(
                                out=y_sb[:], in0=ps[:, 0:OUT_F],
                                in1=y0_sb[:], op=add)
                            hs = range(0, 12)
                        else:
                            hs = range(13 * c - 1, 13 * c + 12)
                        for h in hs:
                            off = 32 + 32 * h - m0
                            nc.vector.scalar_tensor_tensor(
                                out=y_sb[:],
                                in0=ps[:, off:off + 32],
                                scalar=x_sb[:, h:h + 1],
                                in1=y_sb[:],
                                op0=mult,
                                op1=add,
                            )
                    nc.sync.dma_start(out=y[s0:s0 + P, :], in_=y_sb[:])

    nc.finalize()
    return nc


def prep_inputs(x, eps, loc, logvar, cov_flat):
    """Host-side prep: build L^T (padded), eps^T per core (padded), y0."""
    x = np.asarray(x, np.float32)
    eps = np.asarray(eps, np.float32)
    loc = np.asarray(loc, np.float32)
    logvar = np.asarray(logvar, np.float32)
    cov_flat = np.asarray(cov_flat, np.float32)

    n = loc.shape[0]
    assert n == N and x.shape == (B, S, IN_F) and eps.shape == (B, S, N)

    rows, cols = np.tril_indices(n, -1)
    L = np.zeros((n, n), np.float32)
    L[rows, cols] = cov_flat
    L[np.arange(n), np.arange(n)] = np.exp(0.5 * logvar)

    LT_pad = np.zeros((NPAD, n), np.float32)
    LT_pad[:n] = L.T

    epsT_pad = np.zeros((B, NPAD, S), np.float32)
    epsT_pad[:, :n, :] = eps.transpose(0, 2, 1)

    locW = loc[OUT_F:].reshape(IN_F, OUT_F)
    y0 = loc[None, None, :OUT_F] + x @ locW          # [B, S, OUT_F]
    y0 = np.ascontiguousarray(y0, np.float32)

    in_maps = [
        {"epsT": epsT_pad[b], "LT": LT_pad, "x": x[b], "y0": y0[b]}
        for b in range(B)
    ]
    return in_maps


def kernel(x, eps, loc, logvar, cov_flat):
    from concourse.bass_utils import run_bass_kernel_spmd

    in_maps = prep_inputs(x, eps, loc, logvar, cov_flat)
    nc = build_module(use_f32r=True)
    res = run_bass_kernel_spmd(nc, in_maps, list(range(N_CORES)))
    out = np.stack([res.results[b]["y"] for b in range(N_CORES)])
    return out.astype(np.float32)


# revision 28
# speedup vs baseline: 188.6615x; 188.6615x over previous
"""Bayesian linear layer sampling kernel for 8 Trainium2 NeuronCores.

Reference computation (per batch b, sample s):
    L = strictly_lower(cov_flat) + diag(exp(logvar/2))        # [N, N], N=2080
    W[b,s,:] = loc + eps[b,s,:] @ L.T                          # [B,S,N]
    y[b,s,k] = sum_h x[b,s,h] * W[b,s, 32+32h+k] + W[b,s,k]    # [B,S,32]

Strategy:
  - Data parallel over the batch dim: one batch of 2048 samples per core.
  - The loc contribution is affine in x only:
        y0[s,k] = loc[k] + sum_h x[s,h]*loc[32+32h+k]
    so it is precomputed on host and the device computes
        Wg = eps @ L^T
        y  = y0 + Wg[:, :32] + sum_h x[:,h] * Wg[:, 32+32h:64+32h]
  - GEMM on the tensor engine with contraction dim n on partitions:
    stationary lhsT = eps^T tiles [128n x 128s] (eps transposed on host,
    zero padded 2080 -> 2176 = 17*128 rows), moving rhs = L^T tiles
    [128n x 416m].  dtype float32r -> 1 cycle/row on TRN2 for moving dim
    >= 256 (full-rate fp32 matmul).
  - L^T is upper triangular: tile (nt, c) is all-zero when
    416*(c+1) <= 128*nt, so only 51 of 85 (nt, chunk) matmuls per s-tile
    are issued (and only the nonzero column range of L^T is DMA'd).
  - m split into 5 chunks of 416 (>=256, and 416 = 13*32 keeps the
    32-wide per-h weight blocks unsplit across chunks).
  - Epilogue per 128-sample s-tile, on VectorE in 8 wide ops: as each
    chunk finishes accumulating, one broadcast multiply
        tmp[:, h, k] = W_chunk[:, h, k] * x[:, h]
    straight out of PSUM (overlapping the remaining matmuls), then one
    strided reduce over h and two adds for the bias/loc terms.
"""

import sys

if "/opt/trn_rl_repo" not in sys.path:
    sys.path.insert(0, "/opt/trn_rl_repo")

import math
import numpy as np

IN_F, OUT_F = 64, 32
N = IN_F * OUT_F + OUT_F          # 2080
B, S = 8, 2048
P = 128
NT = 17                           # n tiles (2080 padded to 2176 = 17*128)
NPAD = NT * P
M_CHUNK = 416                     # 5 chunks of 416 = 2080
N_CHUNKS = N // M_CHUNK
S_BLK = 512                       # eps^T slab width in s (4 s-tiles)
N_CORES = 8

# tile (nt, c) of L^T is nonzero iff 416*(c+1) > 128*nt
KEEP = [min(NT, math.ceil(M_CHUNK * (c + 1) / P)) for c in range(N_CHUNKS)]
# first chunk each n-tile contributes to (=> L^T cols < 416*c_min are zero)
C_MIN = [min(c for c in range(N_CHUNKS) if KEEP[c] > nt) for nt in range(NT)]
M_LO = [M_CHUNK * c for c in C_MIN]
# eps^T slab DMA groups (parallelism across DMA queues)
EPS_GROUPS = [(0, 5), (5, 9), (9, 13), (13, 17)]


def build_module(use_f32r: bool = True, n_stiles: int = S // P,
                 reps: int = 1, dtype_mode: str | None = None,
                 ablate: str | None = None, lt_in_loop: bool = False):
    """Build the Bass module (one NeuronCore's program, run SPMD on 8).

    reps > 1 wraps the body in a device-side For_i loop recomputing the
    same outputs `reps` times — used only for timing (per-iteration
    slope between two reps values).
    """
    import concourse.bass as bass  # noqa: F401
    import concourse.mybir as mybir
    from concourse import bacc, tile
    from contextlib import nullcontext

    nc = bacc.Bacc("TRN2", target_bir_lowering=False, debug=False,
                   num_devices=N_CORES)
    if dtype_mode is None:
        dtype_mode = "f32r" if use_f32r else "f32"
    mmdt = {"f32r": mybir.dt.float32r, "f32": mybir.dt.float32,
            "bf16": mybir.dt.bfloat16}[dtype_mode]
    f32 = mybir.dt.float32
    SW = S_BLK // P               # s-tiles per eps slab

    epsT = nc.dram_tensor("epsT", [NPAD, S], mmdt, kind="ExternalInput")
    LT = nc.dram_tensor("LT", [NPAD, N], mmdt, kind="ExternalInput")
    x = nc.dram_tensor("x", [S, IN_F], f32, kind="ExternalInput")
    y0 = nc.dram_tensor("y0", [S, OUT_F], f32, kind="ExternalInput")
    y = nc.dram_tensor("y", [S, OUT_F], f32, kind="ExternalOutput")

    # [p, t, *] views of the per-sample tensors (t = s-tile index)
    epsT_r = epsT[:].rearrange("(t p) s -> p t s", p=P)
    x_r = x[:].rearrange("(t p) f -> p t f", p=P)
    y0_r = y0[:].rearrange("(t p) f -> p t f", p=P)
    y_r = y[:].rearrange("(t p) f -> p t f", p=P)

    mult = mybir.AluOpType.mult
    add = mybir.AluOpType.add
    Copy = mybir.ActivationFunctionType.Copy

    with tile.TileContext(nc) as tc:
        with (
            tc.tile_pool(name="lt", bufs=1) as lt_pool,
            tc.tile_pool(name="eps", bufs=2) as eps_pool,
            tc.tile_pool(name="xy", bufs=1) as xy_pool,
            tc.tile_pool(name="yb", bufs=2) as yb_pool,
            tc.tile_pool(name="w", bufs=3) as w_pool,
            tc.tile_pool(name="acc", bufs=4) as acc_pool,
            tc.tile_pool(name="psum", bufs=8, space="PSUM") as psum_pool,
        ):
            # L^T resident for the whole kernel; only the nonzero
            # (upper-triangular) column range of each row-tile is kept.
            lt_sb = [
                lt_pool.tile([P, N - M_LO[nt]], mmdt, tag=f"lt{nt}", bufs=1,
                             name=f"lt{nt}")
                for nt in range(NT)
            ]

            n_sblocks = (n_stiles + SW - 1) // SW

            def load_eps_slab(sb):
                eps_t = eps_pool.tile([P, NT, S_BLK], mmdt, tag="eps")
                for g0, g1 in EPS_GROUPS:
                    nc.sync.dma_start(
                        out=eps_t[:, g0:g1, :],
                        in_=epsT_r[:, g0:g1, sb * S_BLK:(sb + 1) * S_BLK],
                    )
                return eps_t

            def load_lt():
                # c-major so chunk 0's operands land first and the PE can
                # start ~3 us in instead of after the whole L^T load
                for c in range(N_CHUNKS):
                    m0 = c * M_CHUNK
                    for nt in range(KEEP[c]):
                        nc.sync.dma_start(
                            out=lt_sb[nt][:, m0 - M_LO[nt]:
                                          m0 + M_CHUNK - M_LO[nt]],
                            in_=LT[nt * P:(nt + 1) * P, m0:m0 + M_CHUNK],
                        )

            # whole-core x / y0 loads (one DMA each)
            x_all = xy_pool.tile([P, n_stiles, IN_F], f32, tag="x", bufs=1)
            y0_all = xy_pool.tile([P, n_stiles, OUT_F], f32, tag="y0", bufs=1)

            first_eps = None
            if reps == 1:
                # single-shot: queue the first eps slab ahead of the bulk
                # of L^T so neither input blocks the first matmuls
                first_eps = load_eps_slab(0)
                nc.sync.dma_start(out=x_all[:], in_=x_r[:, :n_stiles, :])
                nc.sync.dma_start(out=y0_all[:], in_=y0_r[:, :n_stiles, :])
                load_lt()
            else:
                nc.sync.dma_start(out=x_all[:], in_=x_r[:, :n_stiles, :])
                nc.sync.dma_start(out=y0_all[:], in_=y0_r[:, :n_stiles, :])
                if not lt_in_loop:
                    load_lt()

            rep_ctx = tc.For_i(0, reps, 1) if reps > 1 else nullcontext()
            with rep_ctx:
                for sb in range(n_sblocks):
                    if sb == 0 and first_eps is not None:
                        eps_t = first_eps
                    else:
                        eps_t = load_eps_slab(sb)
                    if sb == 0 and reps > 1 and lt_in_loop:
                        # timing probe: include the L^T load in every
                        # iteration so the slope matches a single-shot run
                        load_lt()
                    sw_eff = min(SW, n_stiles - sb * SW)
                    y_blk = yb_pool.tile([P, SW, OUT_F], f32, tag="y")

                    for half in range(sw_eff):
                        st = sb * SW + half
                        yt = y_blk[:, half, :]
                        pss = [psum_pool.tile([P, M_CHUNK], f32, tag="ps",
                                              name=f"ps{c}")
                               for c in range(N_CHUNKS)]
                        tmp = w_pool.tile([P, IN_F * OUT_F], f32, tag="tmp")
                        t3 = tmp[:].rearrange("p (h k) -> p h k", k=OUT_F)
                        xb3 = x_all[:, st, :, None].broadcast_to(
                            [P, IN_F, OUT_F])

                        # timing-ablation knob (wrong results; never used
                        # by kernel()): "gemm_min" keeps 1 matmul per
                        # chunk, "no_epi" drops the DVE/ACT epilogue
                        keep = ([1] * N_CHUNKS if ablate == "gemm_min"
                                else KEEP)
                        if ablate == "no_epi_nti":
                            # probe: chunk-outer / nt-inner MM ordering
                            for c in range(N_CHUNKS):
                                m0 = c * M_CHUNK
                                for nt in range(KEEP[c]):
                                    nc.tensor.matmul(
                                        pss[c][:],
                                        lhsT=eps_t[:, nt,
                                                   half * P:(half + 1) * P],
                                        rhs=lt_sb[nt][:, m0 - M_LO[nt]:
                                                      m0 + M_CHUNK
                                                      - M_LO[nt]],
                                        start=(nt == 0),
                                        stop=(nt == KEEP[c] - 1),
                                    )
                            nc.scalar.activation(yt, pss[0][:, 0:OUT_F],
                                                 Copy)
                            continue
                        # nt-outer / chunk-inner: 5 consecutive matmuls
                        # share the same stationary eps tile, and chunk c
                        # completes at nt = keep[c]-1 so its ACT staging
                        # overlaps the remaining matmuls of this s-tile
                        for nt in range(NT):
                            for c in range(N_CHUNKS):
                                if keep[c] <= nt:
                                    continue
                                m0 = c * M_CHUNK
                                nc.tensor.matmul(
                                    pss[c][:],
                                    lhsT=eps_t[:, nt,
                                               half * P:(half + 1) * P],
                                    rhs=lt_sb[nt][:, m0 - M_LO[nt]:
                                                  m0 + M_CHUNK - M_LO[nt]],
                                    start=(nt == 0),
                                    stop=(nt == keep[c] - 1),
                                )
                            for c in range(N_CHUNKS):
                                if keep[c] - 1 != nt:
                                    continue
                                if ablate == "no_epi":
                                    if c == 0:
                                        nc.scalar.activation(
                                            yt, pss[c][:, 0:OUT_F], Copy)
                                    continue
                                # chunk c finished accumulating: multiply
                                # its h-blocks by x straight out of PSUM
                                # into the [h, k] staging row (1 wide DVE
                                # op; overlaps the remaining matmuls)
                                if c == 0:
                                    h0, h1, lo = 0, 12, OUT_F
                                    bias = acc_pool.tile([P, OUT_F], f32,
                                                         tag="bias")
                                    nc.vector.tensor_tensor(
                                        out=bias[:], in0=pss[0][:, 0:OUT_F],
                                        in1=y0_all[:, st, :], op=add)
                                else:
                                    h0, h1, lo = 13 * c - 1, 13 * c + 12, 0
                                nc.vector.tensor_tensor(
                                    out=t3[:, h0:h1, :],
                                    in0=pss[c][:, lo:M_CHUNK].rearrange(
                                        "p (h k) -> p h k", k=OUT_F),
                                    in1=xb3[:, h0:h1, :],
                                    op=mult)
                        if ablate != "no_epi":
                            # y[:,k] = bias + sum_h tmp[:,h,k] via one
                            # strided reduce over h + one add
                            red = acc_pool.tile([P, OUT_F], f32, tag="red")
                            nc.vector.tensor_reduce(
                                out=red[:],
                                in_=tmp[:].rearrange("p (h k) -> p k h",
                                                     k=OUT_F),
                                axis=mybir.AxisListType.X, op=add)
                            nc.vector.tensor_tensor(out=yt, in0=bias[:],
                                                    in1=red[:], op=add)
                    nc.sync.dma_start(
                        out=y_r[:, sb * SW:sb * SW + sw_eff, :],
                        in_=y_blk[:, :sw_eff, :])

    nc.finalize()
    return nc


def prep_inputs(x, eps, loc, logvar, cov_flat, dtype_mode: str = "f32r"):
    """Host-side prep: build L^T (padded), eps^T per core (padded), y0."""
    x = np.asarray(x, np.float32)
    eps = np.asarray(eps, np.float32)
    loc = np.asarray(loc, np.float32)
    logvar = np.asarray(logvar, np.float32)
    cov_flat = np.asarray(cov_flat, np.float32)

    n = loc.shape[0]
    assert n == N and x.shape == (B, S, IN_F) and eps.shape == (B, S, N)

    rows, cols = np.tril_indices(n, -1)
    L = np.zeros((n, n), np.float32)
    L[rows, cols] = cov_flat
    L[np.arange(n), np.arange(n)] = np.exp(0.5 * logvar)

    LT_pad = np.zeros((NPAD, n), np.float32)
    LT_pad[:n] = L.T

    epsT_pad = np.zeros((B, NPAD, S), np.float32)
    epsT_pad[:, :n, :] = eps.transpose(0, 2, 1)

    locW = loc[OUT_F:].reshape(IN_F, OUT_F)
    y0 = loc[None, None, :OUT_F] + x @ locW          # [B, S, OUT_F]
    y0 = np.ascontiguousarray(y0, np.float32)

    if dtype_mode == "bf16":
        import ml_dtypes
        LT_pad = LT_pad.astype(ml_dtypes.bfloat16)
        epsT_pad = epsT_pad.astype(ml_dtypes.bfloat16)

    in_maps = [
        {"epsT": epsT_pad[b], "LT": LT_pad, "x": x[b], "y0": y0[b]}
        for b in range(B)
    ]
    return in_maps


def kernel(x, eps, loc, logvar, cov_flat):
    from concourse.bass_utils import run_bass_kernel_spmd

    in_maps = prep_inputs(x, eps, loc, logvar, cov_flat)
    nc = build_module(use_f32r=True)
    res = run_bass_kernel_spmd(nc, in_maps, list(range(N_CORES)))
    out = np.stack([res.results[b]["y"] for b in range(N_CORES)])
    return out.astype(np.float32)


# revision 29
# speedup vs baseline: 189.6171x; 1.0051x over previous
"""Bayesian linear layer sampling kernel for 8 Trainium2 NeuronCores.

Reference computation (per batch b, sample s):
    L = strictly_lower(cov_flat) + diag(exp(logvar/2))        # [N, N], N=2080
    W[b,s,:] = loc + eps[b,s,:] @ L.T                          # [B,S,N]
    y[b,s,k] = sum_h x[b,s,h] * W[b,s, 32+32h+k] + W[b,s,k]    # [B,S,32]

Strategy:
  - Data parallel over the batch dim: one batch of 2048 samples per core.
  - The loc contribution is affine in x only:
        y0[s,k] = loc[k] + sum_h x[s,h]*loc[32+32h+k]
    so it is precomputed on host and the device computes
        Wg = eps @ L^T
        y  = y0 + Wg[:, :32] + sum_h x[:,h] * Wg[:, 32+32h:64+32h]
  - GEMM on the tensor engine with contraction dim n on partitions:
    stationary lhsT = eps^T tiles [128n x 128s] (eps transposed on host,
    zero padded 2080 -> 2176 = 17*128 rows), moving rhs = L^T tiles
    [128n x 416m].  dtype float32r -> 1 cycle/row on TRN2 for moving dim
    >= 256 (full-rate fp32 matmul).
  - L^T is upper triangular: tile (nt, c) is all-zero when
    416*(c+1) <= 128*nt, so only 51 of 85 (nt, chunk) matmuls per s-tile
    are issued (and only the nonzero column range of L^T is DMA'd).
  - m split into 5 chunks of 416 (>=256, and 416 = 13*32 keeps the
    32-wide per-h weight blocks unsplit across chunks).
  - Epilogue per 128-sample s-tile, on VectorE in 8 wide ops: as each
    chunk finishes accumulating, one broadcast multiply
        tmp[:, h, k] = W_chunk[:, h, k] * x[:, h]
    straight out of PSUM (overlapping the remaining matmuls), then one
    strided reduce over h and two adds for the bias/loc terms.
"""

import sys

if "/opt/trn_rl_repo" not in sys.path:
    sys.path.insert(0, "/opt/trn_rl_repo")

import math
import numpy as np

IN_F, OUT_F = 64, 32
N = IN_F * OUT_F + OUT_F          # 2080
B, S = 8, 2048
P = 128
NT = 17                           # n tiles (2080 padded to 2176 = 17*128)
NPAD = NT * P
S_BLK = 512                       # eps^T slab width in s (4 s-tiles)
N_CORES = 8

# m-chunk widths: 32-aligned, in [256, 512] (f32r full-rate needs moving
# dim >= 256); narrow near the diagonal so the triangular skip keeps the
# same 51 matmuls/s-tile at ~5% fewer PE cycles than uniform 416s
CH_W = [256, 256, 256, 384, 512, 416]
CH_END = [sum(CH_W[:c + 1]) for c in range(len(CH_W))]
CH_START = [e - w for e, w in zip(CH_END, CH_W)]
N_CHUNKS = len(CH_W)

# tile (nt, c) of L^T is nonzero iff CH_END[c] > 128*nt
KEEP = [min(NT, math.ceil(CH_END[c] / P)) for c in range(N_CHUNKS)]
# first chunk each n-tile contributes to (=> L^T cols < M_LO[nt] are zero)
C_MIN = [min(c for c in range(N_CHUNKS) if KEEP[c] > nt) for nt in range(NT)]
M_LO = [CH_START[c] for c in C_MIN]
# eps^T slab DMA groups (parallelism across DMA queues)
EPS_GROUPS = [(0, 5), (5, 9), (9, 13), (13, 17)]


def build_module(use_f32r: bool = True, n_stiles: int = S // P,
                 reps: int = 1, dtype_mode: str | None = None,
                 ablate: str | None = None, lt_in_loop: bool = False):
    """Build the Bass module (one NeuronCore's program, run SPMD on 8).

    reps > 1 wraps the body in a device-side For_i loop recomputing the
    same outputs `reps` times — used only for timing (per-iteration
    slope between two reps values).
    """
    import concourse.bass as bass  # noqa: F401
    import concourse.mybir as mybir
    from concourse import bacc, tile
    from contextlib import nullcontext

    nc = bacc.Bacc("TRN2", target_bir_lowering=False, debug=False,
                   num_devices=N_CORES)
    if dtype_mode is None:
        dtype_mode = "f32r" if use_f32r else "f32"
    mmdt = {"f32r": mybir.dt.float32r, "f32": mybir.dt.float32,
            "bf16": mybir.dt.bfloat16}[dtype_mode]
    f32 = mybir.dt.float32
    SW = S_BLK // P               # s-tiles per eps slab

    epsT = nc.dram_tensor("epsT", [NPAD, S], mmdt, kind="ExternalInput")
    LT = nc.dram_tensor("LT", [NPAD, N], mmdt, kind="ExternalInput")
    x = nc.dram_tensor("x", [S, IN_F], f32, kind="ExternalInput")
    y0 = nc.dram_tensor("y0", [S, OUT_F], f32, kind="ExternalInput")
    y = nc.dram_tensor("y", [S, OUT_F], f32, kind="ExternalOutput")

    # [p, t, *] views of the per-sample tensors (t = s-tile index)
    epsT_r = epsT[:].rearrange("(t p) s -> p t s", p=P)
    x_r = x[:].rearrange("(t p) f -> p t f", p=P)
    y0_r = y0[:].rearrange("(t p) f -> p t f", p=P)
    y_r = y[:].rearrange("(t p) f -> p t f", p=P)

    mult = mybir.AluOpType.mult
    add = mybir.AluOpType.add
    Copy = mybir.ActivationFunctionType.Copy

    with tile.TileContext(nc) as tc:
        with (
            tc.tile_pool(name="lt", bufs=1) as lt_pool,
            tc.tile_pool(name="eps", bufs=2) as eps_pool,
            tc.tile_pool(name="xy", bufs=1) as xy_pool,
            tc.tile_pool(name="yb", bufs=2) as yb_pool,
            tc.tile_pool(name="w", bufs=3) as w_pool,
            tc.tile_pool(name="acc", bufs=4) as acc_pool,
            tc.tile_pool(name="psum", bufs=8, space="PSUM") as psum_pool,
        ):
            # L^T resident for the whole kernel; only the nonzero
            # (upper-triangular) column range of each row-tile is kept.
            lt_sb = [
                lt_pool.tile([P, N - M_LO[nt]], mmdt, tag=f"lt{nt}", bufs=1,
                             name=f"lt{nt}")
                for nt in range(NT)
            ]

            n_sblocks = (n_stiles + SW - 1) // SW

            def load_eps_slab(sb):
                eps_t = eps_pool.tile([P, NT, S_BLK], mmdt, tag="eps")
                for g0, g1 in EPS_GROUPS:
                    nc.sync.dma_start(
                        out=eps_t[:, g0:g1, :],
                        in_=epsT_r[:, g0:g1, sb * S_BLK:(sb + 1) * S_BLK],
                    )
                return eps_t

            def load_lt():
                # c-major so chunk 0's operands land first and the PE can
                # start ~3 us in instead of after the whole L^T load
                for c in range(N_CHUNKS):
                    m0, m1 = CH_START[c], CH_END[c]
                    for nt in range(KEEP[c]):
                        nc.sync.dma_start(
                            out=lt_sb[nt][:, m0 - M_LO[nt]:m1 - M_LO[nt]],
                            in_=LT[nt * P:(nt + 1) * P, m0:m1],
                        )

            # whole-core x / y0 loads (one DMA each)
            x_all = xy_pool.tile([P, n_stiles, IN_F], f32, tag="x", bufs=1)
            y0_all = xy_pool.tile([P, n_stiles, OUT_F], f32, tag="y0", bufs=1)

            first_eps = None
            if reps == 1:
                # single-shot: queue the first eps slab ahead of the bulk
                # of L^T so neither input blocks the first matmuls
                first_eps = load_eps_slab(0)
                nc.sync.dma_start(out=x_all[:], in_=x_r[:, :n_stiles, :])
                nc.sync.dma_start(out=y0_all[:], in_=y0_r[:, :n_stiles, :])
                load_lt()
            else:
                nc.sync.dma_start(out=x_all[:], in_=x_r[:, :n_stiles, :])
                nc.sync.dma_start(out=y0_all[:], in_=y0_r[:, :n_stiles, :])
                if not lt_in_loop:
                    load_lt()

            rep_ctx = tc.For_i(0, reps, 1) if reps > 1 else nullcontext()
            with rep_ctx:
                for sb in range(n_sblocks):
                    if sb == 0 and first_eps is not None:
                        eps_t = first_eps
                    else:
                        eps_t = load_eps_slab(sb)
                    if sb == 0 and reps > 1 and lt_in_loop:
                        # timing probe: include the L^T load in every
                        # iteration so the slope matches a single-shot run
                        load_lt()
                    sw_eff = min(SW, n_stiles - sb * SW)
                    y_blk = yb_pool.tile([P, SW, OUT_F], f32, tag="y")

                    for half in range(sw_eff):
                        st = sb * SW + half
                        yt = y_blk[:, half, :]
                        pss = [psum_pool.tile([P, CH_W[c]], f32, tag="ps",
                                              name=f"ps{c}")
                               for c in range(N_CHUNKS)]
                        tmp = w_pool.tile([P, IN_F * OUT_F], f32, tag="tmp")
                        t3 = tmp[:].rearrange("p (h k) -> p h k", k=OUT_F)
                        xb3 = x_all[:, st, :, None].broadcast_to(
                            [P, IN_F, OUT_F])

                        # timing-ablation knob (wrong results; never used
                        # by kernel()): "gemm_min" keeps 1 matmul per
                        # chunk, "no_epi" drops the DVE/ACT epilogue
                        keep = ([1] * N_CHUNKS if ablate == "gemm_min"
                                else KEEP)
                        # nt-outer / chunk-inner: 5 consecutive matmuls
                        # share the same stationary eps tile, and chunk c
                        # completes at nt = keep[c]-1 so its ACT staging
                        # overlaps the remaining matmuls of this s-tile
                        for nt in range(NT):
                            for c in range(N_CHUNKS):
                                if keep[c] <= nt:
                                    continue
                                m0, m1 = CH_START[c], CH_END[c]
                                nc.tensor.matmul(
                                    pss[c][:],
                                    lhsT=eps_t[:, nt,
                                               half * P:(half + 1) * P],
                                    rhs=lt_sb[nt][:, m0 - M_LO[nt]:
                                                  m1 - M_LO[nt]],
                                    start=(nt == 0),
                                    stop=(nt == keep[c] - 1),
                                )
                            for c in range(N_CHUNKS):
                                if keep[c] - 1 != nt:
                                    continue
                                if ablate == "no_epi":
                                    if c == 0:
                                        nc.scalar.activation(
                                            yt, pss[c][:, 0:OUT_F], Copy)
                                    continue
                                # chunk c finished accumulating: multiply
                                # its h-blocks by x straight out of PSUM
                                # into the [h, k] staging row (1 wide DVE
                                # op; overlaps the remaining matmuls)
                                if c == 0:
                                    lo = OUT_F
                                    bias = acc_pool.tile([P, OUT_F], f32,
                                                         tag="bias")
                                    nc.vector.tensor_tensor(
                                        out=bias[:], in0=pss[0][:, 0:OUT_F],
                                        in1=y0_all[:, st, :], op=add)
                                else:
                                    lo = 0
                                h0 = max(0, (CH_START[c] - OUT_F) // 32)
                                h1 = (CH_END[c] - OUT_F) // 32
                                nc.vector.tensor_tensor(
                                    out=t3[:, h0:h1, :],
                                    in0=pss[c][:, lo:CH_W[c]].rearrange(
                                        "p (h k) -> p h k", k=OUT_F),
                                    in1=xb3[:, h0:h1, :],
                                    op=mult)
                        if ablate != "no_epi":
                            # y[:,k] = bias + sum_h tmp[:,h,k] via one
                            # strided reduce over h + one add
                            red = acc_pool.tile([P, OUT_F], f32, tag="red")
                            nc.vector.tensor_reduce(
                                out=red[:],
                                in_=tmp[:].rearrange("p (h k) -> p k h",
                                                     k=OUT_F),
                                axis=mybir.AxisListType.X, op=add)
                            nc.vector.tensor_tensor(out=yt, in0=bias[:],
                                                    in1=red[:], op=add)
                    nc.sync.dma_start(
                        out=y_r[:, sb * SW:sb * SW + sw_eff, :],
                        in_=y_blk[:, :sw_eff, :])

    nc.finalize()
    return nc


def prep_inputs(x, eps, loc, logvar, cov_flat, dtype_mode: str = "f32r"):
    """Host-side prep: build L^T (padded), eps^T per core (padded), y0."""
    x = np.asarray(x, np.float32)
    eps = np.asarray(eps, np.float32)
    loc = np.asarray(loc, np.float32)
    logvar = np.asarray(logvar, np.float32)
    cov_flat = np.asarray(cov_flat, np.float32)

    n = loc.shape[0]
    assert n == N and x.shape == (B, S, IN_F) and eps.shape == (B, S, N)

    rows, cols = np.tril_indices(n, -1)
    L = np.zeros((n, n), np.float32)
    L[rows, cols] = cov_flat
    L[np.arange(n), np.arange(n)] = np.exp(0.5 * logvar)

    LT_pad = np.zeros((NPAD, n), np.float32)
    LT_pad[:n] = L.T

    epsT_pad = np.zeros((B, NPAD, S), np.float32)
    epsT_pad[:, :n, :] = eps.transpose(0, 2, 1)

    locW = loc[OUT_F:].reshape(IN_F, OUT_F)
    y0 = loc[None, None, :OUT_F] + x @ locW          # [B, S, OUT_F]
    y0 = np.ascontiguousarray(y0, np.float32)

    if dtype_mode == "bf16":
        import ml_dtypes
        LT_pad = LT_pad.astype(ml_dtypes.bfloat16)
        epsT_pad = epsT_pad.astype(ml_dtypes.bfloat16)

    in_maps = [
        {"epsT": epsT_pad[b], "LT": LT_pad, "x": x[b], "y0": y0[b]}
        for b in range(B)
    ]
    return in_maps


def kernel(x, eps, loc, logvar, cov_flat):
    from concourse.bass_utils import run_bass_kernel_spmd

    in_maps = prep_inputs(x, eps, loc, logvar, cov_flat)
    nc = build_module(use_f32r=True)
    res = run_bass_kernel_spmd(nc, in_maps, list(range(N_CORES)))
    out = np.stack([res.results[b]["y"] for b in range(N_CORES)])
    return out.astype(np.float32)


# revision 30
# speedup vs baseline: 190.4987x; 1.0046x over previous
"""Bayesian linear layer sampling kernel for 8 Trainium2 NeuronCores.

Reference computation (per batch b, sample s):
    L = strictly_lower(cov_flat) + diag(exp(logvar/2))        # [N, N], N=2080
    W[b,s,:] = loc + eps[b,s,:] @ L.T                          # [B,S,N]
    y[b,s,k] = sum_h x[b,s,h] * W[b,s, 32+32h+k] + W[b,s,k]    # [B,S,32]

Strategy:
  - Data parallel over the batch dim: one batch of 2048 samples per core.
  - The loc contribution is affine in x only:
        y0[s,k] = loc[k] + sum_h x[s,h]*loc[32+32h+k]
    so it is precomputed on host and the device computes
        Wg = eps @ L^T
        y  = y0 + Wg[:, :32] + sum_h x[:,h] * Wg[:, 32+32h:64+32h]
  - GEMM on the tensor engine with contraction dim n on partitions:
    stationary lhsT = eps^T tiles [128n x 128s] (eps transposed on host,
    zero padded 2080 -> 2176 = 17*128 rows), moving rhs = L^T chunk
    tiles.  dtype float32r -> 1 cycle/row on TRN2 for moving dim >= 256
    (full-rate fp32 matmul).
  - L^T is upper triangular: tile (nt, c) is all-zero when
    CH_END[c] <= 128*nt, so only 51 of 102 (nt, chunk) matmuls per
    s-tile are issued (and only the nonzero columns of L^T are DMA'd).
  - m split into width-graded chunks (narrow near the diagonal; all
    >= 256 for full-rate f32r and 32-aligned so the per-h weight blocks
    never straddle a chunk boundary).
  - Epilogue per 128-sample s-tile, on VectorE in 8 wide ops: as each
    chunk finishes accumulating, one broadcast multiply
        tmp[:, h, k] = W_chunk[:, h, k] * x[:, h]
    straight out of PSUM (overlapping the remaining matmuls), then one
    strided reduce over h and two adds for the bias/loc terms.
"""

import sys

if "/opt/trn_rl_repo" not in sys.path:
    sys.path.insert(0, "/opt/trn_rl_repo")

import math
import numpy as np

IN_F, OUT_F = 64, 32
N = IN_F * OUT_F + OUT_F          # 2080
B, S = 8, 2048
P = 128
NT = 17                           # n tiles (2080 padded to 2176 = 17*128)
NPAD = NT * P
S_BLK = 512                       # eps^T slab width in s (4 s-tiles)
N_CORES = 8

# m-chunk widths: 32-aligned, in [256, 512] (f32r full-rate needs moving
# dim >= 256); narrow near the diagonal so the triangular skip keeps the
# same 51 matmuls/s-tile at ~5% fewer PE cycles than uniform 416s
CH_W = [256, 256, 256, 384, 512, 416]
CH_END = [sum(CH_W[:c + 1]) for c in range(len(CH_W))]
CH_START = [e - w for e, w in zip(CH_END, CH_W)]
N_CHUNKS = len(CH_W)

# tile (nt, c) of L^T is nonzero iff CH_END[c] > 128*nt
KEEP = [min(NT, math.ceil(CH_END[c] / P)) for c in range(N_CHUNKS)]
# first chunk each n-tile contributes to (=> L^T cols < M_LO[nt] are zero)
C_MIN = [min(c for c in range(N_CHUNKS) if KEEP[c] > nt) for nt in range(NT)]
M_LO = [CH_START[c] for c in C_MIN]
# eps^T slab DMA groups (parallelism across DMA queues)
EPS_GROUPS = [(0, 5), (5, 9), (9, 13), (13, 17)]


def build_module(use_f32r: bool = True, n_stiles: int = S // P,
                 reps: int = 1, dtype_mode: str | None = None,
                 ablate: str | None = None, lt_in_loop: bool = False):
    """Build the Bass module (one NeuronCore's program, run SPMD on 8).

    reps > 1 wraps the body in a device-side For_i loop recomputing the
    same outputs `reps` times — used only for timing (per-iteration
    slope between two reps values).
    """
    import concourse.bass as bass  # noqa: F401
    import concourse.mybir as mybir
    from concourse import bacc, tile
    from contextlib import nullcontext

    nc = bacc.Bacc("TRN2", target_bir_lowering=False, debug=False,
                   num_devices=N_CORES)
    if dtype_mode is None:
        dtype_mode = "f32r" if use_f32r else "f32"
    mmdt = {"f32r": mybir.dt.float32r, "f32": mybir.dt.float32,
            "bf16": mybir.dt.bfloat16}[dtype_mode]
    f32 = mybir.dt.float32
    SW = S_BLK // P               # s-tiles per eps slab

    epsT = nc.dram_tensor("epsT", [NPAD, S], mmdt, kind="ExternalInput")
    LT = nc.dram_tensor("LT", [NPAD, N], mmdt, kind="ExternalInput")
    x = nc.dram_tensor("x", [S, IN_F], f32, kind="ExternalInput")
    y0 = nc.dram_tensor("y0", [S, OUT_F], f32, kind="ExternalInput")
    y = nc.dram_tensor("y", [S, OUT_F], f32, kind="ExternalOutput")

    # [p, t, *] views of the per-sample tensors (t = s-tile index)
    epsT_r = epsT[:].rearrange("(t p) s -> p t s", p=P)
    x_r = x[:].rearrange("(t p) f -> p t f", p=P)
    y0_r = y0[:].rearrange("(t p) f -> p t f", p=P)
    y_r = y[:].rearrange("(t p) f -> p t f", p=P)

    mult = mybir.AluOpType.mult
    add = mybir.AluOpType.add
    Copy = mybir.ActivationFunctionType.Copy

    with tile.TileContext(nc) as tc:
        with (
            tc.tile_pool(name="lt", bufs=1) as lt_pool,
            tc.tile_pool(name="eps", bufs=2) as eps_pool,
            tc.tile_pool(name="xy", bufs=1) as xy_pool,
            tc.tile_pool(name="yb", bufs=2) as yb_pool,
            tc.tile_pool(name="w", bufs=3) as w_pool,
            tc.tile_pool(name="acc", bufs=4) as acc_pool,
            tc.tile_pool(name="psum", bufs=8, space="PSUM") as psum_pool,
        ):
            # L^T resident for the whole kernel; only the nonzero
            # (upper-triangular) column range of each row-tile is kept.
            lt_sb = [
                lt_pool.tile([P, N - M_LO[nt]], mmdt, tag=f"lt{nt}", bufs=1,
                             name=f"lt{nt}")
                for nt in range(NT)
            ]

            n_sblocks = (n_stiles + SW - 1) // SW

            def load_eps_slab(sb):
                eps_t = eps_pool.tile([P, NT, S_BLK], mmdt, tag="eps")
                for g0, g1 in EPS_GROUPS:
                    nc.sync.dma_start(
                        out=eps_t[:, g0:g1, :],
                        in_=epsT_r[:, g0:g1, sb * S_BLK:(sb + 1) * S_BLK],
                    )
                return eps_t

            def load_lt():
                # c-major so chunk 0's operands land first and the PE can
                # start ~3 us in instead of after the whole L^T load
                for c in range(N_CHUNKS):
                    m0, m1 = CH_START[c], CH_END[c]
                    for nt in range(KEEP[c]):
                        nc.sync.dma_start(
                            out=lt_sb[nt][:, m0 - M_LO[nt]:m1 - M_LO[nt]],
                            in_=LT[nt * P:(nt + 1) * P, m0:m1],
                        )

            # whole-core x / y0 loads (one DMA each)
            x_all = xy_pool.tile([P, n_stiles, IN_F], f32, tag="x", bufs=1)
            y0_all = xy_pool.tile([P, n_stiles, OUT_F], f32, tag="y0", bufs=1)

            first_eps = None
            if reps == 1:
                # single-shot: queue the first eps slab ahead of the bulk
                # of L^T so neither input blocks the first matmuls
                first_eps = load_eps_slab(0)
                nc.sync.dma_start(out=x_all[:], in_=x_r[:, :n_stiles, :])
                nc.sync.dma_start(out=y0_all[:], in_=y0_r[:, :n_stiles, :])
                load_lt()
            else:
                nc.sync.dma_start(out=x_all[:], in_=x_r[:, :n_stiles, :])
                nc.sync.dma_start(out=y0_all[:], in_=y0_r[:, :n_stiles, :])
                if not lt_in_loop:
                    load_lt()

            rep_ctx = tc.For_i(0, reps, 1) if reps > 1 else nullcontext()
            with rep_ctx:
                for sb in range(n_sblocks):
                    if sb == 0 and first_eps is not None:
                        eps_t = first_eps
                    else:
                        eps_t = load_eps_slab(sb)
                    if sb == 0 and reps > 1 and lt_in_loop:
                        # timing probe: include the L^T load in every
                        # iteration so the slope matches a single-shot run
                        load_lt()
                    sw_eff = min(SW, n_stiles - sb * SW)
                    y_blk = yb_pool.tile([P, SW, OUT_F], f32, tag="y")

                    for half in range(sw_eff):
                        st = sb * SW + half
                        yt = y_blk[:, half, :]
                        pss = [psum_pool.tile([P, CH_W[c]], f32, tag="ps",
                                              name=f"ps{c}")
                               for c in range(N_CHUNKS)]
                        tmp = w_pool.tile([P, IN_F * OUT_F], f32, tag="tmp")
                        t3 = tmp[:].rearrange("p (h k) -> p h k", k=OUT_F)
                        xb3 = x_all[:, st, :, None].broadcast_to(
                            [P, IN_F, OUT_F])

                        # timing-ablation knob (wrong results; never used
                        # by kernel()): "gemm_min" keeps 1 matmul per
                        # chunk, "no_epi" drops the DVE/ACT epilogue
                        keep = ([1] * N_CHUNKS if ablate == "gemm_min"
                                else KEEP)
                        # nt-outer / chunk-inner: 5 consecutive matmuls
                        # share the same stationary eps tile, and chunk c
                        # completes at nt = keep[c]-1 so its ACT staging
                        # overlaps the remaining matmuls of this s-tile
                        for nt in range(NT):
                            for c in range(N_CHUNKS):
                                if keep[c] <= nt:
                                    continue
                                m0, m1 = CH_START[c], CH_END[c]
                                nc.tensor.matmul(
                                    pss[c][:],
                                    lhsT=eps_t[:, nt,
                                               half * P:(half + 1) * P],
                                    rhs=lt_sb[nt][:, m0 - M_LO[nt]:
                                                  m1 - M_LO[nt]],
                                    start=(nt == 0),
                                    stop=(nt == keep[c] - 1),
                                )
                            for c in range(N_CHUNKS):
                                if keep[c] - 1 != nt:
                                    continue
                                if ablate == "no_epi":
                                    if c == 0:
                                        nc.scalar.activation(
                                            yt, pss[c][:, 0:OUT_F], Copy)
                                    continue
                                # chunk c finished accumulating: multiply
                                # its h-blocks by x straight out of PSUM
                                # into the [h, k] staging row (1 wide DVE
                                # op; overlaps the remaining matmuls)
                                if c == 0:
                                    lo = OUT_F
                                    bias = acc_pool.tile([P, OUT_F], f32,
                                                         tag="bias")
                                    nc.vector.tensor_tensor(
                                        out=bias[:], in0=pss[0][:, 0:OUT_F],
                                        in1=y0_all[:, st, :], op=add)
                                else:
                                    lo = 0
                                h0 = max(0, (CH_START[c] - OUT_F) // 32)
                                h1 = (CH_END[c] - OUT_F) // 32
                                nc.vector.tensor_tensor(
                                    out=t3[:, h0:h1, :],
                                    in0=pss[c][:, lo:CH_W[c]].rearrange(
                                        "p (h k) -> p h k", k=OUT_F),
                                    in1=xb3[:, h0:h1, :],
                                    op=mult)
                        if ablate != "no_epi":
                            # y[:,k] = bias + sum_h tmp[:,h,k] via one
                            # strided reduce over h + one add
                            red = acc_pool.tile([P, OUT_F], f32, tag="red")
                            nc.vector.tensor_reduce(
                                out=red[:],
                                in_=tmp[:].rearrange("p (h k) -> p k h",
                                                     k=OUT_F),
                                axis=mybir.AxisListType.X, op=add)
                            nc.vector.tensor_tensor(out=yt, in0=bias[:],
                                                    in1=red[:], op=add)
                    nc.sync.dma_start(
                        out=y_r[:, sb * SW:sb * SW + sw_eff, :],
                        in_=y_blk[:, :sw_eff, :])

    nc.finalize()
    return nc


def prep_inputs(x, eps, loc, logvar, cov_flat, dtype_mode: str = "f32r"):
    """Host-side prep: build L^T (padded), eps^T per core (padded), y0."""
    x = np.asarray(x, np.float32)
    eps = np.asarray(eps, np.float32)
    loc = np.asarray(loc, np.float32)
    logvar = np.asarray(logvar, np.float32)
    cov_flat = np.asarray(cov_flat, np.float32)

    n = loc.shape[0]
    assert n == N and x.shape == (B, S, IN_F) and eps.shape == (B, S, N)

    rows, cols = np.tril_indices(n, -1)
    L = np.zeros((n, n), np.float32)
    L[rows, cols] = cov_flat
    L[np.arange(n), np.arange(n)] = np.exp(0.5 * logvar)

    LT_pad = np.zeros((NPAD, n), np.float32)
    LT_pad[:n] = L.T

    epsT_pad = np.zeros((B, NPAD, S), np.float32)
    epsT_pad[:, :n, :] = eps.transpose(0, 2, 1)

    locW = loc[OUT_F:].reshape(IN_F, OUT_F)
    y0 = loc[None, None, :OUT_F] + x @ locW          # [B, S, OUT_F]
    y0 = np.ascontiguousarray(y0, np.float32)

    if dtype_mode == "bf16":
        import ml_dtypes
        LT_pad = LT_pad.astype(ml_dtypes.bfloat16)
        epsT_pad = epsT_pad.astype(ml_dtypes.bfloat16)

    in_maps = [
        {"epsT": epsT_pad[b], "LT": LT_pad, "x": x[b], "y0": y0[b]}
        for b in range(B)
    ]
    return in_maps


def kernel(x, eps, loc, logvar, cov_flat):
    from concourse.bass_utils import run_bass_kernel_spmd

    in_maps = prep_inputs(x, eps, loc, logvar, cov_flat)
    nc = build_module(use_f32r=True)
    res = run_bass_kernel_spmd(nc, in_maps, list(range(N_CORES)))
    out = np.stack([res.results[b]["y"] for b in range(N_CORES)])
    return out.astype(np.float32)
